# revision 6
# baseline (speedup 1.0000x reference)
"""Causal multi-head attention with RoPE on 8 Trainium2 NeuronCores.

Reference computation (fp32):
    qkv = x @ Wqkv.T ; split q,k,v ; heads 16 x 64 ; interleaved-pair RoPE on
    q,k ; causal softmax(q k^T / 8) @ v ; concat heads ; out @ Wout.T

Sharding: core c -> batch b=c//2, head-group g=c%2 (heads 8g..8g+8).
Each core computes a [2048, 1024] partial of the output projection for its
batch (contraction over its 512 head-dims); host sums core pairs.

v3 design (vs v1 baseline at ~408us, v2 at ~377us):
  - Software-pipelined emission: qkv(hp+1) s-chunks and div(hp-1) chunks are
    emitted BETWEEN attention q-chunks of hp, so the in-order PE queue always
    has independent matmuls to run while the exp chain (ACT-bound) catches
    up. proj is emitted per q-chunk after div(3,qc) to pipeline into attn3.
  - Scores for both heads of a pair in one 2-bank psum tile [128, 2, 512];
    ONE exp instruction covers both heads.
  - Causal diag masking by multiplying ex by a 0/1 triangle (gpsimd) after
    exp; PV skips fully-masked columns.
  - V^T computed directly with x^T as the stationary operand (no PE
    transposes).
  - Softmax denominators: gpsimd partition_broadcast of the denom row, DVE
    reciprocal_approx_fast, one multiply per (hp, qc). No PE transposes,
    no PSUM bank, no slow iterative reciprocal.
  - Per-(hp,qc) output tiles so div/proj dependencies are fine-grained.
  - Startup DMAs ordered critical-first (wq0, x chunk 0, tables, rest).
"""

import math
import os
import sys

import numpy as np

sys.path.insert(0, "/opt/trn_rl_repo")

import concourse.bass as bass  # noqa: E402,F401  (re-exported for tooling)
import concourse.mybir as mybir  # noqa: E402
from concourse import bacc, tile  # noqa: E402

D_MODEL = 1024
NUM_HEADS = 16
DH = 64
S = 2048
B = 4
THETA = 10000.0
P = 128
N_CORES = 8
F = 512  # free-dim chunk
N_SC = S // F  # 4 s-chunks
N_QT = S // P  # 16 s-tiles of 128
HPAIRS = 4  # head pairs per core

MM_DT = getattr(mybir.dt, os.environ.get("MM_DT", "bfloat16"))


def build_program(debug: bool = False):
    """Build the single-core SPMD program (identical on all 8 cores)."""
    nc = bacc.Bacc("TRN2", target_bir_lowering=False, debug=debug,
                   enable_asserts=debug)
    f32 = mybir.dt.float32
    cdt = MM_DT
    mult = mybir.AluOpType.mult
    add = mybir.AluOpType.add

    xt_d = nc.dram_tensor("xt", [D_MODEL, S], cdt, kind="ExternalInput")
    wq_d = nc.dram_tensor("wqkv", [D_MODEL, 12 * P], cdt, kind="ExternalInput")
    wo_d = nc.dram_tensor("wout", [4 * P, D_MODEL], cdt, kind="ExternalInput")
    cos_d = nc.dram_tensor("costab", [P, S], cdt, kind="ExternalInput")
    sinw_d = nc.dram_tensor("sinswt", [P, S], cdt, kind="ExternalInput")
    mask_d = nc.dram_tensor("masks", [P, P], cdt, kind="ExternalInput")
    out_d = nc.dram_tensor("out", [S, D_MODEL], f32, kind="ExternalOutput")

    xt_r = xt_d.ap().rearrange("(dc p) s -> p dc s", p=P)  # [128, 8, 2048]
    wq_r = wq_d.ap().rearrange("(dc p) n -> p dc n", p=P)  # [128, 8, 1536]
    wo_r = wo_d.ap().rearrange("(hp p) e -> p hp e", p=P)  # [128, 4, 1024]

    with tile.TileContext(nc) as tc:
        with (
            tc.tile_pool(name="const", bufs=1) as const,
            tc.tile_pool(name="wq", bufs=2) as wqp,
            tc.tile_pool(name="qkv", bufs=2) as qkvp,
            tc.tile_pool(name="tmp", bufs=3) as tmpp,
            tc.tile_pool(name="exp", bufs=6) as expp,
            tc.tile_pool(name="rsr", bufs=8) as rsp,
            tc.tile_pool(name="den", bufs=4) as denp,
            tc.tile_pool(name="rcp", bufs=2) as rcpp,
            tc.tile_pool(name="fin", bufs=4) as finp,
            tc.tile_pool(name="psc", bufs=2, space="PSUM") as scp,
            tc.tile_pool(name="ppo", bufs=2, space="PSUM") as pop,
            tc.tile_pool(name="pmm", bufs=2, space="PSUM") as mmp,
        ):
            # ---- constants & input DMAs, critical-path first ----
            whps = [None] * HPAIRS
            whps[0] = wqp.tile([P, 8, 3 * P], cdt, tag="wq", name="whp0")
            nc.sync.dma_start(whps[0][:], wq_r[:, :, 0:3 * P])
            xts = const.tile([P, 8, S], cdt)
            nc.sync.dma_start(xts[:, :, 0:F], xt_r[:, :, 0:F])
            cost = const.tile([P, S], cdt)
            nc.sync.dma_start(cost[:], cos_d.ap())
            sinw = const.tile([P, S], cdt)
            nc.sync.dma_start(sinw[:], sinw_d.ap())
            tri01 = const.tile([P, P], cdt)
            nc.sync.dma_start(tri01[:], mask_d.ap())
            for sc in range(1, N_SC):
                sl = slice(sc * F, (sc + 1) * F)
                nc.sync.dma_start(xts[:, :, sl], xt_r[:, :, sl])
            woutt = const.tile([P, 4, D_MODEL], cdt)
            nc.sync.dma_start(woutt[:], wo_r)
            onesb = const.tile([P, 64], cdt)
            nc.vector.memset(onesb[:], 1.0)
            # preload the exp table set off the critical path
            dum = const.tile([P, 8], f32)
            nc.scalar.activation(dum[:], onesb[:, 0:8],
                                 mybir.ActivationFunctionType.Exp)
            tri_b = tri01[:, None, :].to_broadcast((P, 2, P))

            # attention numerators, one tile per (head pair, q-chunk):
            # rows = [headA dims | headB dims], cols = 512 q
            outq = [[const.tile([P, F], cdt, name=f"outq{i}_{j}")
                     for j in range(N_SC)] for i in range(HPAIRS)]
            rs_rows = {}  # (hp, qc, h2) -> [1, F] f32 denom row

            # per-hp working tiles, allocated lazily
            qk_tiles = [None] * HPAIRS

            def emit_qkv_chunk(hp, sc):
                """q,k matmuls + RoPE + V^T for one 512-wide s-chunk."""
                if qk_tiles[hp] is None:
                    if whps[hp] is None:
                        whps[hp] = wqp.tile([P, 8, 3 * P], cdt, tag="wq",
                                            name=f"whp{hp}")
                        nc.sync.dma_start(
                            whps[hp][:],
                            wq_r[:, :, hp * 3 * P:(hp + 1) * 3 * P])
                    q_rot = qkvp.tile([P, S], cdt, tag="q_rot")
                    k_rot = qkvp.tile([P, S], cdt, tag="k_rot")
                    v_sb = qkvp.tile([P, N_QT, 130], cdt, tag="v_sb")
                    nc.vector.memset(v_sb[:, :, 64:65], 1.0)
                    nc.vector.memset(v_sb[:, :, 129:130], 1.0)
                    qk_tiles[hp] = (q_rot, k_rot, v_sb)
                whp = whps[hp]
                q_rot, k_rot, v_sb = qk_tiles[hp]
                sl = slice(sc * F, (sc + 1) * F)
                for gi, dst in ((0, q_rot), (1, k_rot)):
                    ps = mmp.tile([P, F], f32, tag="mm", name="psqk")
                    for dc in range(8):
                        nc.tensor.matmul(
                            ps,
                            whp[:, dc, gi * P:(gi + 1) * P],
                            xts[:, dc, sl],
                            start=(dc == 0), stop=(dc == 7),
                        )
                    qk = tmpp.tile([P, F], cdt, tag="qk_sb")
                    nc.vector.tensor_copy(qk[:], ps)
                    # rot = qk*cos + swap_within_head(qk)*sins
                    tcs = tmpp.tile([P, F], cdt, tag="ropetmp")
                    nc.vector.tensor_tensor(tcs[:], qk[:], cost[:, sl], mult)
                    for h2 in (0, 64):
                        nc.vector.tensor_tensor(
                            dst[h2:h2 + 32, sl], qk[h2 + 32:h2 + 64, :],
                            sinw[h2 + 32:h2 + 64, sl], mult)
                        nc.vector.tensor_tensor(
                            dst[h2 + 32:h2 + 64, sl], qk[h2:h2 + 32, :],
                            sinw[h2:h2 + 32, sl], mult)
                    nc.vector.tensor_tensor(dst[:, sl], dst[:, sl],
                                            tcs[:], add)
                # v^T direct: x^T s-tile stationary, Wv chunk moving
                vt = mmp.tile([P, 4, P], f32, tag="mm", name="psvt")
                for j in range(4):
                    st = sc * 4 + j
                    for dc in range(8):
                        nc.tensor.matmul(
                            vt[:, j, :],
                            xts[:, dc, st * P:(st + 1) * P],
                            whp[:, dc, 2 * P:3 * P],
                            start=(dc == 0), stop=(dc == 7),
                        )
                nc.vector.tensor_copy(v_sb[:, sc * 4:sc * 4 + 4, 0:64],
                                      vt[:, :, 0:64])
                nc.vector.tensor_copy(v_sb[:, sc * 4:sc * 4 + 4, 65:129],
                                      vt[:, :, 64:128])

            def emit_attn_qc(hp, qc):
                q_rot, k_rot, v_sb = qk_tiles[hp]
                po = [pop.tile([P, F], f32, tag="po", name=f"po{h2}")
                      for h2 in range(2)]
                nkt = 4 * qc + 4
                LOOKAHEAD = 2

                def emit_scores(kt):
                    lo = max(0, (kt - 4 * qc) * P)
                    sp = scp.tile([P, 2, F], f32, tag="sc", name="sp")
                    for h2 in (0, 1):
                        nc.tensor.matmul(
                            sp[:, h2, lo:F],
                            k_rot[64 * h2:64 * h2 + 64, kt * P:(kt + 1) * P],
                            q_rot[64 * h2:64 * h2 + 64,
                                  qc * F + lo:(qc + 1) * F],
                            start=True, stop=True,
                            skip_group_check=True,
                        )
                    return sp

                pend = {kt: emit_scores(kt)
                        for kt in range(min(LOOKAHEAD, nkt))}
                for kt in range(nkt):
                    lo = max(0, (kt - 4 * qc) * P)
                    diag = kt >= 4 * qc
                    if kt + LOOKAHEAD < nkt:
                        pend[kt + LOOKAHEAD] = emit_scores(kt + LOOKAHEAD)
                    sp = pend.pop(kt)
                    ex = expp.tile([P, 2, F], cdt)
                    nc.scalar.activation(
                        ex[:, :, lo:F], sp[:, :, lo:F],
                        mybir.ActivationFunctionType.Exp,
                        scale=1.0 / math.sqrt(DH))
                    if diag:
                        # zero the non-causal part of the diagonal block
                        nc.vector.tensor_tensor(
                            ex[:, :, lo:lo + P], ex[:, :, lo:lo + P],
                            tri_b, mult)
                    for h2 in (0, 1):
                        nc.tensor.matmul(
                            po[h2][0:65, lo:F],
                            v_sb[:, kt, 65 * h2:65 * h2 + 65],
                            ex[:, h2, lo:F],
                            start=(kt == 0), stop=(kt == nkt - 1),
                            skip_group_check=True,
                        )
                for h2 in (0, 1):
                    nc.vector.tensor_copy(outq[hp][qc][64 * h2:64 * h2 + 64,
                                                       :],
                                          po[h2][0:64, :])
                    rsr = rsp.tile([1, F], f32, tag="rs")
                    nc.vector.tensor_copy(rsr[:], po[h2][64:65, :])
                    rs_rows[(hp, qc, h2)] = rsr

            def emit_div(hp, qc):
                # fast-reciprocal each denom row at partition 0 (the custom
                # DVE op misbehaves at base partition 64), then gpsimd
                # partition-broadcast and one multiply per head
                for h2 in (0, 1):
                    rcp_row = rcpp.tile([1, F], f32, tag="rcp")
                    nc.vector.reciprocal_approx_fast(
                        rcp_row[:], rs_rows[(hp, qc, h2)][:])
                    rb = denp.tile([P, F], f32, tag="den")
                    nc.gpsimd.partition_broadcast(rb[:], rcp_row[:])
                    nc.vector.tensor_tensor(
                        outq[hp][qc][64 * h2:64 * h2 + 64, :],
                        outq[hp][qc][64 * h2:64 * h2 + 64, :],
                        rb[64 * h2:64 * h2 + 64, :], mult)

            def emit_proj_qc(qc):
                for j in range(4):
                    st = qc * 4 + j
                    for ec in range(2):
                        esl = slice(ec * F, (ec + 1) * F)
                        pf = mmp.tile([P, F], f32, tag="mm", name="pf")
                        for hp in range(HPAIRS):
                            nc.tensor.matmul(
                                pf,
                                outq[hp][qc][:, j * P:(j + 1) * P],
                                woutt[:, hp, esl],
                                start=(hp == 0), stop=(hp == 3),
                            )
                        fo = finp.tile([P, F], f32)
                        if (st + ec) % 2 == 0:
                            nc.scalar.copy(fo[:], pf)
                        else:
                            nc.vector.tensor_copy(fo[:], pf)
                        nc.sync.dma_start(
                            out_d.ap()[st * P:(st + 1) * P, esl], fo[:])

            # ---- pipelined emission ----
            _sid = nc.enter_named_scope("qkv0", False)[0]
            for sc in range(N_SC):
                emit_qkv_chunk(0, sc)
            nc.leave_named_scope("qkv0", _sid, False)
            for hp in range(HPAIRS):
                _sid = nc.enter_named_scope(f"attn{hp}", False)[0]
                for qc in range(N_SC):
                    emit_attn_qc(hp, qc)
                    if hp < HPAIRS - 1:
                        emit_qkv_chunk(hp + 1, qc)
                    if hp >= 1:
                        emit_div(hp - 1, qc)
                qk_tiles[hp] = None  # allow pool slot reuse
                nc.leave_named_scope(f"attn{hp}", _sid, False)
            _sid = nc.enter_named_scope("proj", False)[0]
            for qc in range(N_SC):
                emit_div(3, qc)
                emit_proj_qc(qc)
            nc.leave_named_scope("proj", _sid, False)

    nc.compile()
    return nc


def _rope_tables():
    k = np.arange(DH // 2, dtype=np.float64)
    invf = THETA ** (-2.0 * k / DH)
    pos = np.arange(S, dtype=np.float64)
    ang = invf[:, None] * pos[None, :]  # [32, S]
    cos32 = np.cos(ang)
    sin32 = np.sin(ang)
    cos = np.tile(cos32, (4, 1)).astype(np.float32)          # [128, S]
    sins = np.concatenate([-sin32, sin32, -sin32, sin32], 0).astype(np.float32)
    return cos, sins


def _masks():
    # 0/1 upper-inclusive triangle: ex[k_local, q_local] valid iff q >= k
    i = np.arange(P)[:, None]
    j = np.arange(P)[None, :]
    return np.where(i <= j, np.float32(1.0), np.float32(0.0))


def _np_dt():
    if MM_DT == mybir.dt.bfloat16:
        import ml_dtypes
        return np.dtype(ml_dtypes.bfloat16)
    return np.dtype(np.float32)


def host_inputs(x, Wqkv, Wout, core):
    """Per-core input dict (cast to the compute dtype on host)."""
    ndt = _np_dt()
    b, g = core // 2, core % 2
    xt = np.ascontiguousarray(x[b].T).astype(ndt)  # [1024, 2048]
    perm = np.concatenate([np.arange(0, DH, 2), np.arange(1, DH, 2)])
    blocks = []
    for hp in range(HPAIRS):
        hA = 8 * g + 2 * hp
        for off, do_perm in ((0, True), (D_MODEL, True), (2 * D_MODEL, False)):
            for h in (hA, hA + 1):
                rows = Wqkv[off + h * DH: off + (h + 1) * DH]
                if do_perm:
                    rows = rows[perm]
                blocks.append(rows)
    wq = np.ascontiguousarray(np.concatenate(blocks, 0).T).astype(ndt)
    wo = np.ascontiguousarray(Wout[:, 512 * g:512 * (g + 1)].T).astype(ndt)
    cos, sins = _rope_tables()
    return {"xt": xt, "wqkv": wq, "wout": wo,
            "costab": cos.astype(ndt), "sinswt": (-sins).astype(ndt),
            "masks": _masks().astype(ndt)}


_CACHE = {}


def kernel(x, Wqkv, Wout):
    from concourse.bass_utils import run_bass_kernel_spmd

    x = np.asarray(x, dtype=np.float32)
    Wqkv = np.asarray(Wqkv, dtype=np.float32)
    Wout = np.asarray(Wout, dtype=np.float32)

    if "nc" not in _CACHE:
        _CACHE["nc"] = build_program(debug=False)
    nc = _CACHE["nc"]

    in_maps = [host_inputs(x, Wqkv, Wout, c) for c in range(N_CORES)]
    res = run_bass_kernel_spmd(nc, in_maps, list(range(N_CORES))).results
    out = np.empty((B, S, D_MODEL), dtype=np.float32)
    for b in range(B):
        out[b] = res[2 * b]["out"] + res[2 * b + 1]["out"]
    return out


# revision 9
# speedup vs baseline: 1.0306x; 1.0306x over previous
"""Causal multi-head attention with RoPE on 8 Trainium2 NeuronCores.

Reference computation (fp32):
    qkv = x @ Wqkv.T ; split q,k,v ; heads 16 x 64 ; interleaved-pair RoPE on
    q,k ; causal softmax(q k^T / 8) @ v ; concat heads ; out @ Wout.T

Sharding: core c -> batch b=c//2, head-group g=c%2 (heads 8g..8g+8).
Each core computes a [2048, 1024] partial of the output projection for its
batch (contraction over its 512 head-dims); host sums core pairs.

v3 design (vs v1 baseline at ~408us, v2 at ~377us):
  - Software-pipelined emission: qkv(hp+1) s-chunks and div(hp-1) chunks are
    emitted BETWEEN attention q-chunks of hp, so the in-order PE queue always
    has independent matmuls to run while the exp chain (ACT-bound) catches
    up. proj is emitted per q-chunk after div(3,qc) to pipeline into attn3.
  - Scores for both heads of a pair in one 2-bank psum tile [128, 2, 512];
    ONE exp instruction covers both heads.
  - Causal diag masking by multiplying ex by a 0/1 triangle (gpsimd) after
    exp; PV skips fully-masked columns.
  - V^T computed directly with x^T as the stationary operand (no PE
    transposes).
  - Softmax denominators: gpsimd partition_broadcast of the denom row, DVE
    reciprocal_approx_fast, one multiply per (hp, qc). No PE transposes,
    no PSUM bank, no slow iterative reciprocal.
  - Per-(hp,qc) output tiles so div/proj dependencies are fine-grained.
  - Startup DMAs ordered critical-first (wq0, x chunk 0, tables, rest).
"""

import math
import os
import sys

import numpy as np

sys.path.insert(0, "/opt/trn_rl_repo")

import concourse.bass as bass  # noqa: E402,F401  (re-exported for tooling)
import concourse.mybir as mybir  # noqa: E402
from concourse import bacc, tile  # noqa: E402

D_MODEL = 1024
NUM_HEADS = 16
DH = 64
S = 2048
B = 4
THETA = 10000.0
P = 128
N_CORES = 8
F = 512  # free-dim chunk
N_SC = S // F  # 4 s-chunks
N_QT = S // P  # 16 s-tiles of 128
HPAIRS = 4  # head pairs per core

MM_DT = getattr(mybir.dt, os.environ.get("MM_DT", "bfloat16"))


def build_program(debug: bool = False):
    """Build the single-core SPMD program (identical on all 8 cores)."""
    nc = bacc.Bacc("TRN2", target_bir_lowering=False, debug=debug,
                   enable_asserts=debug)
    f32 = mybir.dt.float32
    cdt = MM_DT
    mult = mybir.AluOpType.mult
    add = mybir.AluOpType.add

    xt_d = nc.dram_tensor("xt", [D_MODEL, S], cdt, kind="ExternalInput")
    wq_d = nc.dram_tensor("wqkv", [D_MODEL, 12 * P], cdt, kind="ExternalInput")
    wo_d = nc.dram_tensor("wout", [4 * P, D_MODEL], cdt, kind="ExternalInput")
    cos_d = nc.dram_tensor("costab", [P, S], cdt, kind="ExternalInput")
    sinw_d = nc.dram_tensor("sinswt", [P, S], cdt, kind="ExternalInput")
    mask_d = nc.dram_tensor("masks", [P, P], cdt, kind="ExternalInput")
    out_d = nc.dram_tensor("out", [S, D_MODEL], f32, kind="ExternalOutput")

    xt_r = xt_d.ap().rearrange("(dc p) s -> p dc s", p=P)  # [128, 8, 2048]
    wq_r = wq_d.ap().rearrange("(dc p) n -> p dc n", p=P)  # [128, 8, 1536]
    wo_r = wo_d.ap().rearrange("(hp p) e -> p hp e", p=P)  # [128, 4, 1024]

    with tile.TileContext(nc) as tc:
        with (
            tc.tile_pool(name="const", bufs=1) as const,
            tc.tile_pool(name="wq", bufs=2) as wqp,
            tc.tile_pool(name="qkv", bufs=2) as qkvp,
            tc.tile_pool(name="tmp", bufs=3) as tmpp,
            tc.tile_pool(name="exp", bufs=6) as expp,
            tc.tile_pool(name="den", bufs=4) as denp,
            tc.tile_pool(name="rcp", bufs=8) as rcpp,
            tc.tile_pool(name="fin", bufs=4) as finp,
            tc.tile_pool(name="psc", bufs=2, space="PSUM") as scp,
            tc.tile_pool(name="ppo", bufs=2, space="PSUM") as pop,
            tc.tile_pool(name="pmm", bufs=2, space="PSUM") as mmp,
        ):
            # ---- constants & input DMAs, critical-path first ----
            whps = [None] * HPAIRS
            whps[0] = wqp.tile([P, 8, 3 * P], cdt, tag="wq", name="whp0")
            nc.sync.dma_start(whps[0][:], wq_r[:, :, 0:3 * P])
            XC = 256
            xts = const.tile([P, 8, S], cdt)
            nc.sync.dma_start(xts[:, :, 0:XC], xt_r[:, :, 0:XC])
            cost = const.tile([P, S], cdt)
            nc.sync.dma_start(cost[:], cos_d.ap())
            sinw = const.tile([P, S], cdt)
            nc.sync.dma_start(sinw[:], sinw_d.ap())
            tri01 = const.tile([P, P], cdt)
            nc.sync.dma_start(tri01[:], mask_d.ap())
            for xc in range(1, S // XC):
                sl = slice(xc * XC, (xc + 1) * XC)
                nc.sync.dma_start(xts[:, :, sl], xt_r[:, :, sl])
            woutt = const.tile([P, 4, D_MODEL], cdt)
            nc.sync.dma_start(woutt[:], wo_r)
            onesb = const.tile([P, 64], cdt)
            nc.vector.memset(onesb[:], 1.0)
            # preload the exp table set off the critical path
            dum = const.tile([P, 8], f32)
            nc.scalar.activation(dum[:], onesb[:, 0:8],
                                 mybir.ActivationFunctionType.Exp)
            tri2 = const.tile([P, 2, P], cdt)
            nc.vector.tensor_copy(tri2[:, 0, :], tri01[:])
            nc.vector.tensor_copy(tri2[:, 1, :], tri01[:])

            # attention numerators, one tile per (head pair, q-chunk):
            # rows = [headA dims | headB dims], cols = 512 q
            outq = [[const.tile([P, F], cdt, name=f"outq{i}_{j}")
                     for j in range(N_SC)] for i in range(HPAIRS)]
            rs_rows = {}  # (hp, qc, h2) -> [1, F] f32 denom row

            # per-hp working tiles, allocated lazily
            qk_tiles = [None] * HPAIRS

            def emit_qkv_chunk(hp, sc):
                """q,k matmuls + RoPE + V^T for one 512-wide s-chunk."""
                if qk_tiles[hp] is None:
                    if whps[hp] is None:
                        whps[hp] = wqp.tile([P, 8, 3 * P], cdt, tag="wq",
                                            name=f"whp{hp}")
                        nc.sync.dma_start(
                            whps[hp][:],
                            wq_r[:, :, hp * 3 * P:(hp + 1) * 3 * P])
                    q_rot = qkvp.tile([P, S], cdt, tag="q_rot")
                    k_rot = qkvp.tile([P, S], cdt, tag="k_rot")
                    v_sb = qkvp.tile([P, N_QT, 130], cdt, tag="v_sb")
                    nc.vector.memset(v_sb[:, :, 64:65], 1.0)
                    nc.vector.memset(v_sb[:, :, 129:130], 1.0)
                    qk_tiles[hp] = (q_rot, k_rot, v_sb)
                whp = whps[hp]
                q_rot, k_rot, v_sb = qk_tiles[hp]
                sl = slice(sc * F, (sc + 1) * F)
                for gi, dst in ((0, q_rot), (1, k_rot)):
                    ps = mmp.tile([P, F], f32, tag="mm", name="psqk")
                    for dc in range(8):
                        nc.tensor.matmul(
                            ps,
                            whp[:, dc, gi * P:(gi + 1) * P],
                            xts[:, dc, sl],
                            start=(dc == 0), stop=(dc == 7),
                        )
                    qk = tmpp.tile([P, F], cdt, tag="qk_sb")
                    nc.scalar.copy(qk[:], ps)
                    # rot = qk*cos + swap_within_head(qk)*sins
                    tcs = tmpp.tile([P, F], cdt, tag="ropetmp")
                    nc.vector.tensor_tensor(tcs[:], qk[:], cost[:, sl], mult)
                    for h2 in (0, 64):
                        nc.vector.tensor_tensor(
                            dst[h2:h2 + 32, sl], qk[h2 + 32:h2 + 64, :],
                            sinw[h2 + 32:h2 + 64, sl], mult)
                        nc.vector.tensor_tensor(
                            dst[h2 + 32:h2 + 64, sl], qk[h2:h2 + 32, :],
                            sinw[h2:h2 + 32, sl], mult)
                    nc.vector.tensor_tensor(dst[:, sl], dst[:, sl],
                                            tcs[:], add)
                # v^T direct: x^T s-tile stationary, Wv chunk moving
                vt = mmp.tile([P, 4, P], f32, tag="mm", name="psvt")
                for j in range(4):
                    st = sc * 4 + j
                    for dc in range(8):
                        nc.tensor.matmul(
                            vt[:, j, :],
                            xts[:, dc, st * P:(st + 1) * P],
                            whp[:, dc, 2 * P:3 * P],
                            start=(dc == 0), stop=(dc == 7),
                        )
                nc.vector.tensor_copy(v_sb[:, sc * 4:sc * 4 + 4, 0:64],
                                      vt[:, :, 0:64])
                nc.vector.tensor_copy(v_sb[:, sc * 4:sc * 4 + 4, 65:129],
                                      vt[:, :, 64:128])

            def emit_attn_qc(hp, qc):
                q_rot, k_rot, v_sb = qk_tiles[hp]
                po = [pop.tile([P, F], f32, tag="po", name=f"po{h2}")
                      for h2 in range(2)]
                nkt = 4 * qc + 4
                LOOKAHEAD = 2

                def emit_scores(kt):
                    lo = max(0, (kt - 4 * qc) * P)
                    sp = scp.tile([P, 2, F], f32, tag="sc", name="sp")
                    for h2 in (0, 1):
                        nc.tensor.matmul(
                            sp[:, h2, lo:F],
                            k_rot[64 * h2:64 * h2 + 64, kt * P:(kt + 1) * P],
                            q_rot[64 * h2:64 * h2 + 64,
                                  qc * F + lo:(qc + 1) * F],
                            start=True, stop=True,
                            skip_group_check=True,
                        )
                    return sp

                pend = {kt: emit_scores(kt)
                        for kt in range(min(LOOKAHEAD, nkt))}
                for kt in range(nkt):
                    lo = max(0, (kt - 4 * qc) * P)
                    diag = kt >= 4 * qc
                    if kt + LOOKAHEAD < nkt:
                        pend[kt + LOOKAHEAD] = emit_scores(kt + LOOKAHEAD)
                    sp = pend.pop(kt)
                    ex = expp.tile([P, 2, F], cdt)
                    nc.scalar.activation(
                        ex[:, :, lo:F], sp[:, :, lo:F],
                        mybir.ActivationFunctionType.Exp,
                        scale=1.0 / math.sqrt(DH))
                    if diag:
                        # zero the non-causal part of the diagonal block
                        nc.vector.tensor_tensor(
                            ex[:, :, lo:lo + P], ex[:, :, lo:lo + P],
                            tri2[:], mult)
                    for h2 in (0, 1):
                        nc.tensor.matmul(
                            po[h2][0:65, lo:F],
                            v_sb[:, kt, 65 * h2:65 * h2 + 65],
                            ex[:, h2, lo:F],
                            start=(kt == 0), stop=(kt == nkt - 1),
                            skip_group_check=True,
                        )
                for h2 in (0, 1):
                    nc.vector.tensor_copy(outq[hp][qc][64 * h2:64 * h2 + 64,
                                                       :],
                                          po[h2][0:64, :])
                    rsr = rcpp.tile([1, F], f32, tag="rs")
                    nc.scalar.copy(rsr[:], po[h2][64:65, :])
                    rs_rows[(hp, qc, h2)] = rsr

            def emit_div(hp, qc):
                # reciprocal, gpsimd partition-broadcast, multiply per head
                for h2 in (0, 1):
                    rcp_row = rcpp.tile([1, F], f32, tag="rcp")
                    nc.vector.reciprocal_approx_fast(
                        rcp_row[:], rs_rows[(hp, qc, h2)][:])
                    rb = denp.tile([P, F], f32, tag="den")
                    nc.gpsimd.partition_broadcast(rb[:], rcp_row[:])
                    nc.vector.tensor_tensor(
                        outq[hp][qc][64 * h2:64 * h2 + 64, :],
                        outq[hp][qc][64 * h2:64 * h2 + 64, :],
                        rb[64 * h2:64 * h2 + 64, :], mult)

            def emit_proj_qc(qc):
                for j in range(4):
                    st = qc * 4 + j
                    for ec in range(2):
                        esl = slice(ec * F, (ec + 1) * F)
                        pf = mmp.tile([P, F], f32, tag="mm", name="pf")
                        for hp in range(HPAIRS):
                            nc.tensor.matmul(
                                pf,
                                outq[hp][qc][:, j * P:(j + 1) * P],
                                woutt[:, hp, esl],
                                start=(hp == 0), stop=(hp == 3),
                            )
                        fo = finp.tile([P, F], f32)
                        nc.scalar.copy(fo[:], pf)
                        nc.sync.dma_start(
                            out_d.ap()[st * P:(st + 1) * P, esl], fo[:])

            # ---- pipelined emission ----
            _sid = nc.enter_named_scope("qkv0", False)[0]
            for sc in range(N_SC):
                emit_qkv_chunk(0, sc)
            nc.leave_named_scope("qkv0", _sid, False)
            for hp in range(HPAIRS):
                _sid = nc.enter_named_scope(f"attn{hp}", False)[0]
                for qc in range(N_SC):
                    emit_attn_qc(hp, qc)
                    if hp < HPAIRS - 1:
                        emit_qkv_chunk(hp + 1, qc)
                    if hp >= 1:
                        emit_div(hp - 1, qc)
                qk_tiles[hp] = None  # allow pool slot reuse
                nc.leave_named_scope(f"attn{hp}", _sid, False)
            _sid = nc.enter_named_scope("proj", False)[0]
            for qc in range(N_SC):
                emit_div(3, qc)
                emit_proj_qc(qc)
            nc.leave_named_scope("proj", _sid, False)

    nc.compile()
    return nc


def _rope_tables():
    k = np.arange(DH // 2, dtype=np.float64)
    invf = THETA ** (-2.0 * k / DH)
    pos = np.arange(S, dtype=np.float64)
    ang = invf[:, None] * pos[None, :]  # [32, S]
    cos32 = np.cos(ang)
    sin32 = np.sin(ang)
    cos = np.tile(cos32, (4, 1)).astype(np.float32)          # [128, S]
    sins = np.concatenate([-sin32, sin32, -sin32, sin32], 0).astype(np.float32)
    return cos, sins


def _masks():
    # 0/1 upper-inclusive triangle: ex[k_local, q_local] valid iff q >= k
    i = np.arange(P)[:, None]
    j = np.arange(P)[None, :]
    return np.where(i <= j, np.float32(1.0), np.float32(0.0))


def _np_dt():
    if MM_DT == mybir.dt.bfloat16:
        import ml_dtypes
        return np.dtype(ml_dtypes.bfloat16)
    return np.dtype(np.float32)


def host_inputs(x, Wqkv, Wout, core):
    """Per-core input dict (cast to the compute dtype on host)."""
    ndt = _np_dt()
    b, g = core // 2, core % 2
    xt = np.ascontiguousarray(x[b].T).astype(ndt)  # [1024, 2048]
    perm = np.concatenate([np.arange(0, DH, 2), np.arange(1, DH, 2)])
    blocks = []
    for hp in range(HPAIRS):
        hA = 8 * g + 2 * hp
        for off, do_perm in ((0, True), (D_MODEL, True), (2 * D_MODEL, False)):
            for h in (hA, hA + 1):
                rows = Wqkv[off + h * DH: off + (h + 1) * DH]
                if do_perm:
                    rows = rows[perm]
                blocks.append(rows)
    wq = np.ascontiguousarray(np.concatenate(blocks, 0).T).astype(ndt)
    wo = np.ascontiguousarray(Wout[:, 512 * g:512 * (g + 1)].T).astype(ndt)
    cos, sins = _rope_tables()
    return {"xt": xt, "wqkv": wq, "wout": wo,
            "costab": cos.astype(ndt), "sinswt": (-sins).astype(ndt),
            "masks": _masks().astype(ndt)}


_CACHE = {}


def kernel(x, Wqkv, Wout):
    from concourse.bass_utils import run_bass_kernel_spmd

    x = np.asarray(x, dtype=np.float32)
    Wqkv = np.asarray(Wqkv, dtype=np.float32)
    Wout = np.asarray(Wout, dtype=np.float32)

    if "nc" not in _CACHE:
        _CACHE["nc"] = build_program(debug=False)
    nc = _CACHE["nc"]

    in_maps = [host_inputs(x, Wqkv, Wout, c) for c in range(N_CORES)]
    res = run_bass_kernel_spmd(nc, in_maps, list(range(N_CORES))).results
    out = np.empty((B, S, D_MODEL), dtype=np.float32)
    for b in range(B):
        out[b] = res[2 * b]["out"] + res[2 * b + 1]["out"]
    return out
